# revision 49
# baseline (speedup 1.0000x reference)
"""Causal dilated conv1d (K=3, dilation=2, N=128 channels) on Trainium2.

out[b,t,i] = sum_{j,k} x[b, t-2k, j] * weight[i,j,k] + bias[i]

Strategy (8-core SPMD, pure data parallel over batch, bf16 datapath):
  - each core handles 4 of the 32 batch rows; weight/bias replicated
  - host casts x/weight to bf16 (tolerance is 2e-2; bf16 path lands ~3e-3)
  - per batch row, a [128(j), T+16] bf16 "strip" is filled directly by
    xbar DMA-transpose loads (HBM [t,j] -> SBUF [j,t]); issue alternates
    between the two HWDGE rings (sync and scalar engines) because the
    ucode descriptor generation (~4us per 1MB) would serialize on one
  - 3 accumulated bf16 matmuls per 512-wide window compute
    out_T[i,t] = sum_k w_k^T @ strip[:, t-2k], accumulating into a bf16
    PSUM tile so the drain reads 16-bit; ACT/DVE split the drain that
    adds bias
  - PE transposes out_T back to [t,i] layout (one chunk delayed so the
    PE never stalls on drains), stores are bf16 with 8KB-contiguous
    per-partition pieces at half-chunk granularity; host upcasts to fp32
"""

import threading

import ml_dtypes
import numpy as np

import concourse.bass as bass  # noqa: F401  (bass types used via bacc/tile)
import concourse.mybir as mybir
import concourse.tile as tile
from concourse import bacc
from concourse.bass_utils import run_bass_kernel_spmd
from concourse.masks import make_identity

P = 128
KTAPS = 3
DIL = 2
HALO = 16  # only the last (KTAPS-1)*DIL = 4 cols are read; 16 keeps 32B align
NCORES = 8
B_FULL, T_FULL = 32, 8192
B_CORE = B_FULL // NCORES  # 4

FP32 = mybir.dt.float32
BF16 = mybir.dt.bfloat16


def build(Bc=B_CORE, T=T_FULL, chunk=4096, acc_dtype=FP32):
    """Build the per-core Bass module. Same NEFF runs SPMD on all 8 cores."""
    nc = bacc.Bacc(
        "TRN2",
        target_bir_lowering=False,
        debug=False,
        enable_asserts=False,
        num_devices=NCORES,
    )
    x_d = nc.dram_tensor("x", [Bc, T, P], BF16, kind="ExternalInput")
    w_d = nc.dram_tensor("w", [P, KTAPS * P], BF16, kind="ExternalInput")
    b_d = nc.dram_tensor("b", [P, 1], FP32, kind="ExternalInput")
    o_d = nc.dram_tensor("o", [Bc, T, P], BF16, kind="ExternalOutput")

    x_ap, o_ap = x_d.ap(), o_d.ap()
    n_chunks = T // chunk
    SW = 512  # tap-matmul moving width (1 PSUM bank of fp32)
    S = chunk // SW  # strips per chunk
    GP = SW // P  # 128-wide transposes per tout group
    R = chunk // P  # out rows per partition in the contiguous store

    with tile.TileContext(nc) as tc:
        with (
            tc.tile_pool(name="const", bufs=1) as cp,
            tc.tile_pool(name="strip", bufs=4) as sp,
            tc.tile_pool(name="oT", bufs=3) as otp,
            tc.tile_pool(name="oc", bufs=3) as ocp,
            tc.tile_pool(name="pacc", bufs=4, space="PSUM") as paccp,
            tc.tile_pool(name="pto", bufs=3, space="PSUM") as ptop,
        ):
            ident = cp.tile([P, P], FP32)
            make_identity(nc, ident)
            ident_b = cp.tile([P, P], BF16)
            nc.vector.tensor_copy(ident_b[:], ident[:])
            w_sb = cp.tile([P, KTAPS * P], BF16)
            nc.sync.dma_start(w_sb[:], w_d.ap())
            bias_sb = cp.tile([P, 1], FP32)
            nc.sync.dma_start(bias_sb[:], b_d.ap())


            # one-chunk-delayed transpose-out state
            pending = None
            oc_pending = None

            def emit_tout_group(g):
                nonlocal oc_pending
                oTv_p, b_p, t0_p = pending
                if g == 0:
                    oc_pending = ocp.tile([P, chunk], BF16, tag="oc")
                pto = ptop.tile([P, SW], BF16, tag="pto")
                for c in range(GP):
                    r = g * GP + c
                    nc.tensor.transpose(
                        pto[:, c * P : (c + 1) * P], oTv_p[:, r, :], ident_b
                    )
                dst = oc_pending[:, g * SW : (g + 1) * SW]
                if g % 2 == 0:
                    nc.scalar.copy(dst, pto[:])
                else:
                    nc.vector.tensor_copy(dst, pto[:])

            def emit_out_dma():
                _, b_p, t0_p = pending
                nc.sync.dma_start(
                    o_ap[b_p, t0_p : t0_p + chunk, :].rearrange(
                        "(p f) j -> p (f j)", p=P
                    ),
                    oc_pending[:],
                )

            # two-row interleaved chunk order: each chunk's xbar load gets
            # ~2 chunk-computes (~13us) of other-row work to hide behind
            # instead of ~7us, so the PE stops starving on load latency
            order = []
            for pair in range(0, Bc, 2):
                for ci in range(n_chunks):
                    order.append((pair, ci))
                    order.append((pair + 1, ci))
            # -> (0,0),(1,0),(0,1),(1,1),(2,0),(3,0),(2,1),(3,1)

            strips = {}

            def emit_load(idx):
                if idx >= len(order):
                    return
                b, ci = order[idx]
                t0 = ci * chunk
                if ci == 0:
                    strip = sp.tile([P, T + HALO], BF16, tag="strip")
                    nc.vector.memset(strip[:, 0:HALO], 0.0)
                    strips[b] = strip
                strip = strips[b]
                dst = strip[:, HALO + t0 : HALO + t0 + chunk]
                src = x_ap[b, t0 : t0 + chunk, :]
                if idx == 0:
                    for q in range(4):
                        qw = chunk // 4
                        nc.sync.dma_start(
                            dst[:, q * qw : (q + 1) * qw],
                            src[q * qw : (q + 1) * qw, :],
                            transpose=True,
                        )
                else:
                    nc.sync.dma_start(dst, src, transpose=True)

            emit_load(0)
            emit_load(1)
            for idx, (b, ci) in enumerate(order):
                t0 = ci * chunk
                emit_load(idx + 2)
                strip = strips[b]
                if True:
                    # out_T accumulator for the whole chunk: [i, t-t0]
                    oT = otp.tile([P, chunk], BF16, tag="oT")
                    for s in range(S):
                        st = t0 + s * SW
                        # --- 3 dilated taps, accumulated in PSUM ---
                        pacc = paccp.tile([P, SW], acc_dtype, tag="pacc")
                        for k in range(KTAPS):
                            off = HALO + st - DIL * k
                            nc.tensor.matmul(
                                pacc[:],
                                w_sb[:, k * P : (k + 1) * P],
                                strip[:, off : off + SW],
                                start=(k == 0),
                                stop=(k == KTAPS - 1),
                            )
                        # --- bias + cast to bf16 during the PSUM drain ---
                        dst_oT = oT[:, s * SW : (s + 1) * SW]
                        if s % 2 == 0:
                            nc.scalar.add(dst_oT, pacc[:], bias_sb[:])
                        else:
                            nc.vector.tensor_scalar_add(dst_oT, pacc[:], bias_sb[:])
                        # --- delayed transpose-out of the PREVIOUS chunk ---
                        if pending is not None:
                            emit_tout_group(s)
                    if pending is not None:
                        emit_out_dma()
                    pending = (oT.rearrange("n (p r) -> n r p", p=P), b, t0)
            # epilogue: restore + store the final chunk
            for g in range(S):
                emit_tout_group(g)
            emit_out_dma()
    nc.compile()
    return nc


_cache = {}
_lock = threading.Lock()


def _get_nc():
    with _lock:
        if "nc" not in _cache:
            _cache["nc"] = build()
        return _cache["nc"]


def prep_inputs(x, weight, bias):
    # w_all[j, k*128 + i] = weight[i, j, k]
    w_all = np.ascontiguousarray(
        np.transpose(np.asarray(weight, np.float32), (1, 2, 0)).reshape(P, KTAPS * P)
    ).astype(ml_dtypes.bfloat16)
    b2 = np.ascontiguousarray(np.asarray(bias, np.float32).reshape(P, 1))
    xb = np.asarray(x, np.float32).astype(ml_dtypes.bfloat16)
    return np.ascontiguousarray(xb), w_all, b2


def kernel(x, weight, bias, _trace=False):
    x, w_all, b2 = prep_inputs(x, weight, bias)
    nc = _get_nc()
    in_maps = [
        {"x": x[c * B_CORE : (c + 1) * B_CORE], "w": w_all, "b": b2}
        for c in range(NCORES)
    ]
    res = run_bass_kernel_spmd(nc, in_maps, core_ids=list(range(NCORES)), trace=_trace)
    out = np.concatenate([r["o"] for r in res.results], axis=0).astype(np.float32)
    if _trace:
        kernel.last_results = res
    return out
